# revision 25
# baseline (speedup 1.0000x reference)
"""Trainium2 Bass kernel for single-query attention + output projection.

Math (per batch b):
    s   = ctx[b] @ o[b]               # (2048,)  scores
    a   = softmax(s)                  # (2048,)  attn  (output #2)
    mix = a @ ctx[b]                  # (1024,)
    out = tanh(W @ [mix; o[b]] + bias)  # (1024,)  (output #1)

Sharding: data-parallel over batch. 8 cores x 8 batches each.
Single streaming pass over context (memory-bound roofline).

Per-core pipeline:
  - ctx[b] streamed as 16 tiles [128(n), 1024(d)] (native layout).
  - scores via DVE tensor_tensor_reduce (fused mul + free-dim reduce)
    against the host-replicated query o_rep[b] [128, 1024].
  - softmax: global max via PE transpose + DVE reduce + PE bcast-matmul;
    exp on ACT with fused accum_out row-sums; partition-sum via PE
    ones-matmul; reciprocal on DVE.
  - mix via PE: ctx tile [128(n), 128-d-slice] as stationary, attn column
    [128, 1] as moving -> psum [128(d), 1], accumulated over n-tiles.
    Result lands directly in combined^T layout.
  - final projection batched over the core's 8 batches:
    out[8, 1024] = tanh(combT^T @ WT + 1 x bias_row), with the bias folded
    in as an extra K=1 matmul. WT = W.T is pre-transposed on the host.

All constants (identity for PE transpose, ones vectors) are shipped from
the host; no gpsimd ops, no rearranged DRAM access patterns.
"""

import sys
from contextlib import ExitStack

import numpy as np

sys.path.insert(0, "/opt/trn_rl_repo")

import concourse.bass as bass
import concourse.tile as tile
from concourse import bacc, mybir
from concourse._compat import with_exitstack
from concourse.bass_utils import run_bass_kernel_spmd

F32 = mybir.dt.float32
AX = mybir.AluOpType
AF = mybir.ActivationFunctionType

N_CORES = 8
B, N, D = 64, 2048, 1024
BP = B // N_CORES          # batches per core = 8
NT = N // 128              # n tiles per batch = 16
DC = D // 128              # d chunks = 8
EC = 2 * D // 128          # e chunks for projection = 16


@with_exitstack
def attn_kernel(ctx: ExitStack, tc: tile.TileContext,
                ctx_d, orep_d, oT_d, WT_d, brow_d, ident_d, onesr_d, onesc_d,
                out_d, attn_d):
    nc = tc.nc

    const_pool = ctx.enter_context(tc.tile_pool(name="const", bufs=1))
    wt_pool = ctx.enter_context(tc.tile_pool(name="wt", bufs=6))
    ctx_pool = ctx.enter_context(tc.tile_pool(name="ctx", bufs=24))
    orep_pool = ctx.enter_context(tc.tile_pool(name="orep", bufs=2))
    sm_pool = ctx.enter_context(tc.tile_pool(name="sm", bufs=3))
    comb_pool = ctx.enter_context(tc.tile_pool(name="comb", bufs=1))
    scratch_pool = ctx.enter_context(tc.tile_pool(name="scratch", bufs=2))
    out_pool = ctx.enter_context(tc.tile_pool(name="outp", bufs=1))

    ps_small = ctx.enter_context(tc.tile_pool(name="ps_small", bufs=2, space="PSUM"))
    ps_mix = ctx.enter_context(tc.tile_pool(name="ps_mix", bufs=2, space="PSUM"))
    ps_proj = ctx.enter_context(tc.tile_pool(name="ps_proj", bufs=1, space="PSUM"))

    # --- constants from host ---
    ident = const_pool.tile([128, 128], F32)
    nc.sync.dma_start(out=ident[:], in_=ident_d[:, :])
    ones_row = const_pool.tile([1, 128], F32)
    nc.sync.dma_start(out=ones_row[:], in_=onesr_d[:, :])
    ones_col = const_pool.tile([128, 1], F32)
    nc.sync.dma_start(out=ones_col[:], in_=onesc_d[:, :])
    brow = const_pool.tile([1, D], F32)
    nc.sync.dma_start(out=brow[:], in_=brow_d[:, :])

    # combined^T [128, EC, BP]: e-chunks 0..7 = mix^T, 8..15 = o^T
    combT = comb_pool.tile([128, EC, BP], F32)
    for c in range(DC):
        nc.sync.dma_start(out=combT[:, DC + c, :], in_=oT_d[c])

    for b in range(BP):
        orep = orep_pool.tile([128, D], F32, tag="orep")
        nc.sync.dma_start(out=orep[:], in_=orep_d[b])

        # --- scores: S[p, t] = sum_d ctx[128t+p, d] * o[d] ---
        S = sm_pool.tile([128, NT], F32, tag="S")
        ctx_tiles = []
        for t in range(NT):
            ct = ctx_pool.tile([128, D], F32, tag="ctx")
            nc.sync.dma_start(out=ct[:], in_=ctx_d[b, bass.ts(t, 128), :])
            ctx_tiles.append(ct)
            scr = scratch_pool.tile([128, D], F32, tag="scr")
            nc.vector.tensor_tensor(out=scr[:], in0=ct[:], in1=orep[:],
                                    op=AX.mult)
            scr2 = scratch_pool.tile([128, D], F32, tag="scr2")
            nc.scalar.activation(scr2[:], scr[:], AF.Identity,
                                 accum_out=S[:, t:t + 1])

        # --- softmax over all 2048 scores ---
        m1 = sm_pool.tile([128, 1], F32, tag="m1")
        nc.vector.reduce_max(m1[:], S[:], axis=mybir.AxisListType.X)
        ps_tr = ps_small.tile([1, 128], F32, tag="ps_small")
        nc.tensor.transpose(ps_tr[:], m1[:], ident[:])
        mm = sm_pool.tile([1, 1], F32, tag="mm")
        nc.vector.reduce_max(mm[:], ps_tr[:], axis=mybir.AxisListType.X)
        ps_bc = ps_small.tile([128, 1], F32, tag="ps_small")
        nc.tensor.matmul(ps_bc[:], ones_row[:], mm[:], start=True, stop=True)
        mneg = sm_pool.tile([128, 1], F32, tag="mneg")
        nc.scalar.mul(mneg[:], ps_bc[:], -1.0)

        P = sm_pool.tile([128, NT], F32, tag="P")
        rs = sm_pool.tile([128, 1], F32, tag="rs")
        nc.scalar.activation(P[:], S[:], AF.Exp, bias=mneg[:], scale=1.0,
                             accum_out=rs[:])
        ps_sum = ps_small.tile([1, 1], F32, tag="ps_small")
        nc.tensor.matmul(ps_sum[:], ones_col[:], rs[:], start=True, stop=True)
        inv = sm_pool.tile([1, 1], F32, tag="inv")
        nc.vector.reciprocal(inv[:], ps_sum[:])
        ps_bi = ps_small.tile([128, 1], F32, tag="ps_small")
        nc.tensor.matmul(ps_bi[:], ones_row[:], inv[:], start=True, stop=True)
        inv_col = sm_pool.tile([128, 1], F32, tag="inv_col")
        nc.vector.tensor_copy(inv_col[:], ps_bi[:])

        A = sm_pool.tile([128, NT], F32, tag="A")
        nc.vector.tensor_scalar_mul(A[:], P[:], inv_col[:])

        # --- attn output: transpose [128, 16] -> [16, 128] for contiguous DMA ---
        ps_at = ps_small.tile([NT, 128], F32, tag="ps_small")
        nc.tensor.transpose(ps_at[:], A[:], ident[:])
        At = sm_pool.tile([NT, 128], F32, tag="At")
        nc.vector.tensor_copy(At[:], ps_at[:])
        nc.sync.dma_start(out=attn_d[b], in_=At[:])

        # --- mix: psum_row[1, 512] += A[:, t]^T @ ctx_tile half ---
        # attn column as stationary (tiny LDW); context as wide moving.
        psr = [ps_mix.tile([1, 512], F32, tag=f"psr{h}", name=f"psr{h}")
               for h in range(2)]
        for t in range(NT):
            for h in range(2):
                nc.tensor.matmul(psr[h][:], A[:, t:t + 1],
                                 ctx_tiles[t][:, bass.ts(h, 512)],
                                 start=(t == 0), stop=(t == NT - 1))
        mix_row = sm_pool.tile([1, D], F32, tag="mix_row")
        for h in range(2):
            nc.vector.tensor_copy(mix_row[:, bass.ts(h, 512)], psr[h][:])
        # row -> combT columns via PE transpose of [1, 128] slices
        for c in range(DC):
            ps_mc = ps_small.tile([128, 1], F32, tag="ps_small")
            nc.tensor.transpose(ps_mc[:], mix_row[:, bass.ts(c, 128)],
                                ident[0:1, 0:1])
            nc.vector.tensor_copy(combT[:, c:c + 1, b], ps_mc[:])

    # --- projection: out[8, 1024] = tanh(combT^T @ WT + 1 x brow) ---
    # WT streamed (not resident); each tile feeds both output halves.
    out_sb = out_pool.tile([BP, D], F32, tag="out_sb")
    ps_ps = [ps_proj.tile([BP, 512], F32, tag=f"ps_p{h}", name=f"ps_p{h}")
             for h in range(2)]
    for e in range(EC):
        wt = wt_pool.tile([128, D], F32, tag="wt")
        nc.sync.dma_start(out=wt[:], in_=WT_d[bass.ts(e, 128), :])
        for h in range(2):
            nc.tensor.matmul(ps_ps[h][:], combT[:, e, :],
                             wt[:, bass.ts(h, 512)],
                             start=(e == 0), stop=False)
    for h in range(2):
        nc.tensor.matmul(ps_ps[h][:], ones_row[:1, 0:BP],
                         brow[:, bass.ts(h, 512)], start=False, stop=True)
        nc.scalar.activation(out_sb[:, bass.ts(h, 512)], ps_ps[h][:], AF.Tanh)
    nc.sync.dma_start(out=out_d[:, :], in_=out_sb[:])


def build_program():
    nc = bacc.Bacc("TRN2", target_bir_lowering=False, debug=False,
                   enable_asserts=False)
    ctx_d = nc.dram_tensor("ctx", [BP, N, D], F32, kind="ExternalInput").ap()
    orep_d = nc.dram_tensor("orep", [BP, 128, D], F32, kind="ExternalInput").ap()
    oT_d = nc.dram_tensor("oT", [DC, 128, BP], F32, kind="ExternalInput").ap()
    WT_d = nc.dram_tensor("WT", [2 * D, D], F32, kind="ExternalInput").ap()
    brow_d = nc.dram_tensor("brow", [1, D], F32, kind="ExternalInput").ap()
    ident_d = nc.dram_tensor("ident", [128, 128], F32, kind="ExternalInput").ap()
    onesr_d = nc.dram_tensor("onesr", [1, 128], F32, kind="ExternalInput").ap()
    onesc_d = nc.dram_tensor("onesc", [128, 1], F32, kind="ExternalInput").ap()
    out_d = nc.dram_tensor("out", [BP, D], F32, kind="ExternalOutput").ap()
    attn_d = nc.dram_tensor("attn", [BP, NT, 128], F32, kind="ExternalOutput").ap()

    with tile.TileContext(nc) as tc:
        attn_kernel(tc, ctx_d, orep_d, oT_d, WT_d, brow_d, ident_d, onesr_d,
                    onesc_d, out_d, attn_d)
    nc.finalize()
    return nc


_prog_cache = {}


def _get_program():
    if "nc" not in _prog_cache:
        _prog_cache["nc"] = build_program()
    return _prog_cache["nc"]


def make_in_maps(output, context, W_out, b_out):
    o2 = np.ascontiguousarray(np.asarray(output).reshape(B, D)).astype(np.float32)
    WT = np.ascontiguousarray(np.asarray(W_out).T).astype(np.float32)
    brow = np.ascontiguousarray(np.asarray(b_out).reshape(1, D)).astype(np.float32)
    ident = np.eye(128, dtype=np.float32)
    onesr = np.ones((1, 128), np.float32)
    onesc = np.ones((128, 1), np.float32)
    in_maps = []
    for i in range(N_CORES):
        sl = slice(i * BP, (i + 1) * BP)
        osh = o2[sl]  # [BP, D]
        in_maps.append({
            "ctx": np.ascontiguousarray(np.asarray(context)[sl], dtype=np.float32),
            "orep": np.ascontiguousarray(
                np.broadcast_to(osh[:, None, :], (BP, 128, D))),
            "oT": np.ascontiguousarray(
                osh.T.reshape(DC, 128, BP)),
            "WT": WT,
            "brow": brow,
            "ident": ident,
            "onesr": onesr,
            "onesc": onesc,
        })
    return in_maps


def kernel(output, context, W_out, b_out):
    nc = _get_program()
    in_maps = make_in_maps(output, context, W_out, b_out)
    res = run_bass_kernel_spmd(nc, in_maps, list(range(N_CORES))).results
    out_full = np.empty((B, 1, D), np.float32)
    attn_full = np.empty((B, 1, N), np.float32)
    for i in range(N_CORES):
        sl = slice(i * BP, (i + 1) * BP)
        out_full[sl, 0, :] = res[i]["out"]
        attn_full[sl, 0, :] = res[i]["attn"].reshape(BP, N)
    return (out_full, attn_full)


if __name__ == "__main__":
    np.random.seed(0)
    o = np.random.randn(B, 1, D).astype(np.float32)
    c = np.random.randn(B, N, D).astype(np.float32)
    W = np.random.randn(D, 2 * D).astype(np.float32) / np.sqrt(2 * D)
    bb = np.zeros(D, np.float32)
    out, attn = kernel(output=o, context=c, W_out=W, b_out=bb)
    print(out.shape, attn.shape, out.dtype, attn.dtype)


# revision 27
# speedup vs baseline: 1.1571x; 1.1571x over previous
"""Trainium2 Bass kernel for single-query attention + output projection.

Math (per batch b):
    s   = ctx[b] @ o[b]               # (2048,)  scores
    a   = softmax(s)                  # (2048,)  attn  (output #2)
    mix = a @ ctx[b]                  # (1024,)
    out = tanh(W @ [mix; o[b]] + bias)  # (1024,)  (output #1)

Sharding: data-parallel over batch. 8 cores x 8 batches each.
Single streaming pass over context (memory-bound roofline).

Per-core pipeline:
  - ctx[b] streamed as 16 tiles [128(n), 1024(d)] (native layout).
  - scores via DVE tensor_tensor_reduce (fused mul + free-dim reduce)
    against the host-replicated query o_rep[b] [128, 1024].
  - softmax: global max via PE transpose + DVE reduce + PE bcast-matmul;
    exp on ACT with fused accum_out row-sums; partition-sum via PE
    ones-matmul; reciprocal on DVE.
  - mix via PE: ctx tile [128(n), 128-d-slice] as stationary, attn column
    [128, 1] as moving -> psum [128(d), 1], accumulated over n-tiles.
    Result lands directly in combined^T layout.
  - final projection batched over the core's 8 batches:
    out[8, 1024] = tanh(combT^T @ WT + 1 x bias_row), with the bias folded
    in as an extra K=1 matmul. WT = W.T is pre-transposed on the host.

All constants (identity for PE transpose, ones vectors) are shipped from
the host; no gpsimd ops, no rearranged DRAM access patterns.
"""

import sys
from contextlib import ExitStack

import numpy as np

sys.path.insert(0, "/opt/trn_rl_repo")

import concourse.bass as bass
import concourse.tile as tile
from concourse import bacc, mybir
from concourse._compat import with_exitstack
from concourse.bass_utils import run_bass_kernel_spmd

F32 = mybir.dt.float32
AX = mybir.AluOpType
AF = mybir.ActivationFunctionType

N_CORES = 8
B, N, D = 64, 2048, 1024
BP = B // N_CORES          # batches per core = 8
NT = N // 128              # n tiles per batch = 16
DC = D // 128              # d chunks = 8
EC = 2 * D // 128          # e chunks for projection = 16


@with_exitstack
def attn_kernel(ctx: ExitStack, tc: tile.TileContext,
                ctx_d, orep_d, oT_d, WT_d, brow_d, ident_d, onesr_d, onesc_d,
                out_d, attn_d):
    nc = tc.nc

    const_pool = ctx.enter_context(tc.tile_pool(name="const", bufs=1))
    wt_pool = ctx.enter_context(tc.tile_pool(name="wt", bufs=6))
    ctx_pool = ctx.enter_context(tc.tile_pool(name="ctx", bufs=24))
    orep_pool = ctx.enter_context(tc.tile_pool(name="orep", bufs=2))
    sm_pool = ctx.enter_context(tc.tile_pool(name="sm", bufs=3))
    comb_pool = ctx.enter_context(tc.tile_pool(name="comb", bufs=1))
    scratch_pool = ctx.enter_context(tc.tile_pool(name="scratch", bufs=2))
    out_pool = ctx.enter_context(tc.tile_pool(name="outp", bufs=1))

    ps_small = ctx.enter_context(tc.tile_pool(name="ps_small", bufs=2, space="PSUM"))
    ps_mix = ctx.enter_context(tc.tile_pool(name="ps_mix", bufs=2, space="PSUM"))
    ps_proj = ctx.enter_context(tc.tile_pool(name="ps_proj", bufs=1, space="PSUM"))

    # --- constants from host ---
    ident = const_pool.tile([128, 128], F32)
    nc.sync.dma_start(out=ident[:], in_=ident_d[:, :])
    ones_row = const_pool.tile([1, 128], F32)
    nc.sync.dma_start(out=ones_row[:], in_=onesr_d[:, :])
    ones_col = const_pool.tile([128, 1], F32)
    nc.sync.dma_start(out=ones_col[:], in_=onesc_d[:, :])
    brow = const_pool.tile([1, D], F32)
    nc.sync.dma_start(out=brow[:], in_=brow_d[:, :])

    # combined^T [128, EC, BP]: e-chunks 0..7 = mix^T, 8..15 = o^T
    combT = comb_pool.tile([128, EC, BP], F32)
    for c in range(DC):
        nc.sync.dma_start(out=combT[:, DC + c, :], in_=oT_d[c])

    for b in range(BP):
        orep = orep_pool.tile([128, D], F32, tag="orep")
        nc.sync.dma_start(out=orep[:], in_=orep_d[b])

        # --- scores: S[p, t] = sum_d ctx[128t+p, d] * o[d] ---
        S = sm_pool.tile([128, NT], F32, tag="S")
        ctx_tiles = []
        for t in range(NT):
            ct = ctx_pool.tile([128, D], F32, tag="ctx")
            nc.sync.dma_start(out=ct[:], in_=ctx_d[b, bass.ts(t, 128), :])
            ctx_tiles.append(ct)
            scr = scratch_pool.tile([128, D], F32, tag="scr")
            nc.vector.tensor_tensor(out=scr[:], in0=ct[:], in1=orep[:],
                                    op=AX.mult)
            scr2 = scratch_pool.tile([128, D], F32, tag="scr2")
            nc.scalar.activation(scr2[:], scr[:], AF.Identity,
                                 accum_out=S[:, t:t + 1])

        # --- softmax over all 2048 scores ---
        m1 = sm_pool.tile([128, 1], F32, tag="m1")
        nc.vector.reduce_max(m1[:], S[:], axis=mybir.AxisListType.X)
        ps_tr = ps_small.tile([1, 128], F32, tag="ps_small")
        nc.tensor.transpose(ps_tr[:], m1[:], ident[:])
        mm = sm_pool.tile([1, 1], F32, tag="mm")
        nc.vector.reduce_max(mm[:], ps_tr[:], axis=mybir.AxisListType.X)
        ps_bc = ps_small.tile([128, 1], F32, tag="ps_small")
        nc.tensor.matmul(ps_bc[:], ones_row[:], mm[:], start=True, stop=True)
        mneg = sm_pool.tile([128, 1], F32, tag="mneg")
        nc.scalar.mul(mneg[:], ps_bc[:], -1.0)

        P = sm_pool.tile([128, NT], F32, tag="P")
        rs = sm_pool.tile([128, 1], F32, tag="rs")
        nc.scalar.activation(P[:], S[:], AF.Exp, bias=mneg[:], scale=1.0,
                             accum_out=rs[:])
        ps_sum = ps_small.tile([1, 1], F32, tag="ps_small")
        nc.tensor.matmul(ps_sum[:], ones_col[:], rs[:], start=True, stop=True)

        # --- mix immediately on UNNORMALIZED exp weights P (1/Z folded in
        # at the PSUM->SBUF copy): keeps PE busy while the sum/reciprocal/
        # broadcast chain and the attn-output path run on other engines.
        psr = [ps_mix.tile([1, 512], F32, tag=f"psr{h}", name=f"psr{h}")
               for h in range(2)]
        for t in range(NT):
            for h in range(2):
                nc.tensor.matmul(psr[h][:], P[:, t:t + 1],
                                 ctx_tiles[t][:, bass.ts(h, 512)],
                                 start=(t == 0), stop=(t == NT - 1))

        inv = sm_pool.tile([1, 1], F32, tag="inv")
        nc.vector.reciprocal(inv[:], ps_sum[:])
        ps_bi = ps_small.tile([128, 1], F32, tag="ps_small")
        nc.tensor.matmul(ps_bi[:], ones_row[:], inv[:], start=True, stop=True)
        inv_col = sm_pool.tile([128, 1], F32, tag="inv_col")
        nc.scalar.copy(inv_col[:], ps_bi[:])

        A = sm_pool.tile([128, NT], F32, tag="A")
        nc.vector.tensor_scalar_mul(A[:], P[:], inv_col[:])

        # --- attn output: transpose [128, 16] -> [16, 128] for contiguous DMA ---
        ps_at = ps_small.tile([NT, 128], F32, tag="ps_small")
        nc.tensor.transpose(ps_at[:], A[:], ident[:])
        At = sm_pool.tile([NT, 128], F32, tag="At")
        nc.scalar.copy(At[:], ps_at[:])
        nc.sync.dma_start(out=attn_d[b], in_=At[:])

        mix_row = sm_pool.tile([1, D], F32, tag="mix_row")
        for h in range(2):
            nc.vector.tensor_scalar_mul(mix_row[:, bass.ts(h, 512)],
                                        psr[h][:], inv[:])
        # row -> combT columns via PE transpose of [1, 128] slices
        for c in range(DC):
            ps_mc = ps_small.tile([128, 1], F32, tag="ps_small")
            nc.tensor.transpose(ps_mc[:], mix_row[:, bass.ts(c, 128)],
                                ident[0:1, 0:1])
            nc.scalar.copy(combT[:, c:c + 1, b], ps_mc[:])

    # --- projection: out[8, 1024] = tanh(combT^T @ WT + 1 x brow) ---
    # WT streamed (not resident); each tile feeds both output halves.
    out_sb = out_pool.tile([BP, D], F32, tag="out_sb")
    ps_ps = [ps_proj.tile([BP, 512], F32, tag=f"ps_p{h}", name=f"ps_p{h}")
             for h in range(2)]
    for e in range(EC):
        wt = wt_pool.tile([128, D], F32, tag="wt")
        nc.sync.dma_start(out=wt[:], in_=WT_d[bass.ts(e, 128), :])
        for h in range(2):
            nc.tensor.matmul(ps_ps[h][:], combT[:, e, :],
                             wt[:, bass.ts(h, 512)],
                             start=(e == 0), stop=False)
    for h in range(2):
        nc.tensor.matmul(ps_ps[h][:], ones_row[:1, 0:BP],
                         brow[:, bass.ts(h, 512)], start=False, stop=True)
        nc.scalar.activation(out_sb[:, bass.ts(h, 512)], ps_ps[h][:], AF.Tanh)
    nc.sync.dma_start(out=out_d[:, :], in_=out_sb[:])


def build_program():
    nc = bacc.Bacc("TRN2", target_bir_lowering=False, debug=False,
                   enable_asserts=False)
    ctx_d = nc.dram_tensor("ctx", [BP, N, D], F32, kind="ExternalInput").ap()
    orep_d = nc.dram_tensor("orep", [BP, 128, D], F32, kind="ExternalInput").ap()
    oT_d = nc.dram_tensor("oT", [DC, 128, BP], F32, kind="ExternalInput").ap()
    WT_d = nc.dram_tensor("WT", [2 * D, D], F32, kind="ExternalInput").ap()
    brow_d = nc.dram_tensor("brow", [1, D], F32, kind="ExternalInput").ap()
    ident_d = nc.dram_tensor("ident", [128, 128], F32, kind="ExternalInput").ap()
    onesr_d = nc.dram_tensor("onesr", [1, 128], F32, kind="ExternalInput").ap()
    onesc_d = nc.dram_tensor("onesc", [128, 1], F32, kind="ExternalInput").ap()
    out_d = nc.dram_tensor("out", [BP, D], F32, kind="ExternalOutput").ap()
    attn_d = nc.dram_tensor("attn", [BP, NT, 128], F32, kind="ExternalOutput").ap()

    with tile.TileContext(nc) as tc:
        attn_kernel(tc, ctx_d, orep_d, oT_d, WT_d, brow_d, ident_d, onesr_d,
                    onesc_d, out_d, attn_d)
    nc.finalize()
    return nc


_prog_cache = {}


def _get_program():
    if "nc" not in _prog_cache:
        _prog_cache["nc"] = build_program()
    return _prog_cache["nc"]


def make_in_maps(output, context, W_out, b_out):
    o2 = np.ascontiguousarray(np.asarray(output).reshape(B, D)).astype(np.float32)
    WT = np.ascontiguousarray(np.asarray(W_out).T).astype(np.float32)
    brow = np.ascontiguousarray(np.asarray(b_out).reshape(1, D)).astype(np.float32)
    ident = np.eye(128, dtype=np.float32)
    onesr = np.ones((1, 128), np.float32)
    onesc = np.ones((128, 1), np.float32)
    in_maps = []
    for i in range(N_CORES):
        sl = slice(i * BP, (i + 1) * BP)
        osh = o2[sl]  # [BP, D]
        in_maps.append({
            "ctx": np.ascontiguousarray(np.asarray(context)[sl], dtype=np.float32),
            "orep": np.ascontiguousarray(
                np.broadcast_to(osh[:, None, :], (BP, 128, D))),
            "oT": np.ascontiguousarray(
                osh.T.reshape(DC, 128, BP)),
            "WT": WT,
            "brow": brow,
            "ident": ident,
            "onesr": onesr,
            "onesc": onesc,
        })
    return in_maps


def kernel(output, context, W_out, b_out):
    nc = _get_program()
    in_maps = make_in_maps(output, context, W_out, b_out)
    res = run_bass_kernel_spmd(nc, in_maps, list(range(N_CORES))).results
    out_full = np.empty((B, 1, D), np.float32)
    attn_full = np.empty((B, 1, N), np.float32)
    for i in range(N_CORES):
        sl = slice(i * BP, (i + 1) * BP)
        out_full[sl, 0, :] = res[i]["out"]
        attn_full[sl, 0, :] = res[i]["attn"].reshape(BP, N)
    return (out_full, attn_full)


if __name__ == "__main__":
    np.random.seed(0)
    o = np.random.randn(B, 1, D).astype(np.float32)
    c = np.random.randn(B, N, D).astype(np.float32)
    W = np.random.randn(D, 2 * D).astype(np.float32) / np.sqrt(2 * D)
    bb = np.zeros(D, np.float32)
    out, attn = kernel(output=o, context=c, W_out=W, b_out=bb)
    print(out.shape, attn.shape, out.dtype, attn.dtype)


# revision 30
# speedup vs baseline: 1.1919x; 1.0301x over previous
"""Trainium2 Bass kernel for single-query attention + output projection.

Math (per batch b):
    s   = ctx[b] @ o[b]               # (2048,)  scores
    a   = softmax(s)                  # (2048,)  attn  (output #2)
    mix = a @ ctx[b]                  # (1024,)
    out = tanh(W @ [mix; o[b]] + bias)  # (1024,)  (output #1)

Sharding: data-parallel over batch. 8 cores x 8 batches each.
Single streaming pass over context (memory-bound roofline).

Per-core pipeline:
  - ctx[b] streamed as 16 tiles [128(n), 1024(d)] (native layout).
  - scores: DVE elementwise multiply against the host-replicated query
    o_rep[b] [128, 1024], free-dim reduce on ACT via
    activation(Identity, accum_out=...). (tensor_tensor_reduce would fuse
    both, but that instruction hangs the device on this runtime.)
  - softmax: global max via PE transpose + DVE reduce + PE bcast-matmul;
    exp on ACT with fused accum_out row-sums; partition-sum via PE
    ones-matmul; reciprocal on DVE.
  - mix via PE on UNNORMALIZED exp weights P, started right after the
    exp: P column [128, 1] as stationary (tiny LDW), ctx tile halves
    [128, 512] as wide fp32 moving operand -> psum rows [1, 512],
    accumulated over n-tiles; 1/Z is folded into the PSUM->SBUF drain.
    (fp32 matmuls lower to 2 passes; a large fp32 stationary per tile
    costs ~4x more PE time than this moving-form.)
  - mix row -> combined^T columns via [1, 128] PE transposes.
  - final projection batched over the core's 8 batches:
    out[8, 1024] = tanh(combT^T @ WT + 1 x bias_row), with the bias folded
    in as an extra K=1 matmul. WT = W.T is pre-transposed on the host.

All constants (identity for PE transpose, ones vectors) are shipped from
the host; no gpsimd ops, no rearranged DRAM access patterns.
"""

import sys
from contextlib import ExitStack

import numpy as np

sys.path.insert(0, "/opt/trn_rl_repo")

import concourse.bass as bass
import concourse.tile as tile
from concourse import bacc, mybir
from concourse._compat import with_exitstack
from concourse.bass_utils import run_bass_kernel_spmd

F32 = mybir.dt.float32
AX = mybir.AluOpType
AF = mybir.ActivationFunctionType

N_CORES = 8
B, N, D = 64, 2048, 1024
BP = B // N_CORES          # batches per core = 8
NT = N // 128              # n tiles per batch = 16
DC = D // 128              # d chunks = 8
EC = 2 * D // 128          # e chunks for projection = 16


@with_exitstack
def attn_kernel(ctx: ExitStack, tc: tile.TileContext,
                ctx_d, orep_d, oT_d, WT_d, brow_d, ident_d, onesr_d, onesc_d,
                out_d, attn_d):
    nc = tc.nc

    const_pool = ctx.enter_context(tc.tile_pool(name="const", bufs=1))
    wt_pool = ctx.enter_context(tc.tile_pool(name="wt", bufs=6))
    ctx_pool = ctx.enter_context(tc.tile_pool(name="ctx", bufs=24))
    orep_pool = ctx.enter_context(tc.tile_pool(name="orep", bufs=2))
    sm_pool = ctx.enter_context(tc.tile_pool(name="sm", bufs=3))
    comb_pool = ctx.enter_context(tc.tile_pool(name="comb", bufs=1))
    scratch_pool = ctx.enter_context(tc.tile_pool(name="scratch", bufs=2))
    out_pool = ctx.enter_context(tc.tile_pool(name="outp", bufs=1))

    ps_small = ctx.enter_context(tc.tile_pool(name="ps_small", bufs=2, space="PSUM"))
    ps_mix = ctx.enter_context(tc.tile_pool(name="ps_mix", bufs=2, space="PSUM"))
    ps_proj = ctx.enter_context(tc.tile_pool(name="ps_proj", bufs=1, space="PSUM"))

    # --- constants from host ---
    ident = const_pool.tile([128, 128], F32)
    nc.sync.dma_start(out=ident[:], in_=ident_d[:, :])
    ones_row = const_pool.tile([1, 128], F32)
    nc.sync.dma_start(out=ones_row[:], in_=onesr_d[:, :])
    ones_col = const_pool.tile([128, 1], F32)
    nc.sync.dma_start(out=ones_col[:], in_=onesc_d[:, :])
    brow = const_pool.tile([1, D], F32)
    nc.sync.dma_start(out=brow[:], in_=brow_d[:, :])

    # combined^T [128, EC, BP]: e-chunks 0..7 = mix^T, 8..15 = o^T
    combT = comb_pool.tile([128, EC, BP], F32)
    for c in range(DC):
        nc.sync.dma_start(out=combT[:, DC + c, :], in_=oT_d[c])

    # Projection PSUM opens early: the o^T half of combT is ready at
    # program start, so its 8 e-chunks accumulate one-per-batch inside
    # the loop (filling PE idle gaps); only the mix half runs in the tail.
    ps_ps = [ps_proj.tile([BP, 512], F32, tag=f"ps_p{h}", name=f"ps_p{h}")
             for h in range(2)]

    for b in range(BP):
        e_early = DC + b
        wt_e = wt_pool.tile([128, D], F32, tag="wt")
        nc.sync.dma_start(out=wt_e[:], in_=WT_d[bass.ts(e_early, 128), :])
        for h in range(2):
            nc.tensor.matmul(ps_ps[h][:], combT[:, e_early, :],
                             wt_e[:, bass.ts(h, 512)],
                             start=(b == 0), stop=False)
        orep = orep_pool.tile([128, D], F32, tag="orep")
        nc.sync.dma_start(out=orep[:], in_=orep_d[b])

        # --- scores: S[p, t] = sum_d ctx[128t+p, d] * o[d] ---
        S = sm_pool.tile([128, NT], F32, tag="S")
        ctx_tiles = []
        for t in range(NT):
            ct = ctx_pool.tile([128, D], F32, tag="ctx")
            nc.sync.dma_start(out=ct[:], in_=ctx_d[b, bass.ts(t, 128), :])
            ctx_tiles.append(ct)
            scr = scratch_pool.tile([128, D], F32, tag="scr")
            nc.vector.tensor_tensor(out=scr[:], in0=ct[:], in1=orep[:],
                                    op=AX.mult)
            scr2 = scratch_pool.tile([128, D], F32, tag="scr2")
            nc.scalar.activation(scr2[:], scr[:], AF.Identity,
                                 accum_out=S[:, t:t + 1])

        # --- softmax over all 2048 scores ---
        m1 = sm_pool.tile([128, 1], F32, tag="m1")
        nc.vector.reduce_max(m1[:], S[:], axis=mybir.AxisListType.X)
        ps_tr = ps_small.tile([1, 128], F32, tag="ps_small")
        nc.tensor.transpose(ps_tr[:], m1[:], ident[:])
        mm = sm_pool.tile([1, 1], F32, tag="mm")
        nc.vector.reduce_max(mm[:], ps_tr[:], axis=mybir.AxisListType.X)
        ps_bc = ps_small.tile([128, 1], F32, tag="ps_small")
        nc.tensor.matmul(ps_bc[:], ones_row[:], mm[:], start=True, stop=True)
        mneg = sm_pool.tile([128, 1], F32, tag="mneg")
        nc.scalar.mul(mneg[:], ps_bc[:], -1.0)

        P = sm_pool.tile([128, NT], F32, tag="P")
        rs = sm_pool.tile([128, 1], F32, tag="rs")
        nc.scalar.activation(P[:], S[:], AF.Exp, bias=mneg[:], scale=1.0,
                             accum_out=rs[:])
        ps_sum = ps_small.tile([1, 1], F32, tag="ps_small")
        nc.tensor.matmul(ps_sum[:], ones_col[:], rs[:], start=True, stop=True)

        # --- mix immediately on UNNORMALIZED exp weights P (1/Z folded in
        # at the PSUM->SBUF copy): keeps PE busy while the sum/reciprocal/
        # broadcast chain and the attn-output path run on other engines.
        psr = [ps_mix.tile([1, 512], F32, tag=f"psr{h}", name=f"psr{h}")
               for h in range(2)]
        for t in range(NT):
            for h in range(2):
                nc.tensor.matmul(psr[h][:], P[:, t:t + 1],
                                 ctx_tiles[t][:, bass.ts(h, 512)],
                                 start=(t == 0), stop=(t == NT - 1))

        inv = sm_pool.tile([1, 1], F32, tag="inv")
        nc.vector.reciprocal(inv[:], ps_sum[:])
        ps_bi = ps_small.tile([128, 1], F32, tag="ps_small")
        nc.tensor.matmul(ps_bi[:], ones_row[:], inv[:], start=True, stop=True)
        inv_col = sm_pool.tile([128, 1], F32, tag="inv_col")
        nc.scalar.copy(inv_col[:], ps_bi[:])

        A = sm_pool.tile([128, NT], F32, tag="A")
        nc.vector.tensor_scalar_mul(A[:], P[:], inv_col[:])

        # --- attn output: transpose [128, 16] -> [16, 128] for contiguous DMA ---
        ps_at = ps_small.tile([NT, 128], F32, tag="ps_small")
        nc.tensor.transpose(ps_at[:], A[:], ident[:])
        At = sm_pool.tile([NT, 128], F32, tag="At")
        nc.scalar.copy(At[:], ps_at[:])
        nc.sync.dma_start(out=attn_d[b], in_=At[:])

        mix_row = sm_pool.tile([1, D], F32, tag="mix_row")
        for h in range(2):
            nc.vector.tensor_scalar_mul(mix_row[:, bass.ts(h, 512)],
                                        psr[h][:], inv[:])
        # row -> combT columns via PE transpose of [1, 128] slices
        for c in range(DC):
            ps_mc = ps_small.tile([128, 1], F32, tag="ps_small")
            nc.tensor.transpose(ps_mc[:], mix_row[:, bass.ts(c, 128)],
                                ident[0:1, 0:1])
            nc.scalar.copy(combT[:, c:c + 1, b], ps_mc[:])

    # --- projection tail: mix-part e-chunks + bias, then tanh ---
    out_sb = out_pool.tile([BP, D], F32, tag="out_sb")
    for e in range(DC):
        wt = wt_pool.tile([128, D], F32, tag="wt")
        nc.sync.dma_start(out=wt[:], in_=WT_d[bass.ts(e, 128), :])
        for h in range(2):
            nc.tensor.matmul(ps_ps[h][:], combT[:, e, :],
                             wt[:, bass.ts(h, 512)],
                             start=False, stop=False)
    for h in range(2):
        nc.tensor.matmul(ps_ps[h][:], ones_row[:1, 0:BP],
                         brow[:, bass.ts(h, 512)], start=False, stop=True)
        nc.scalar.activation(out_sb[:, bass.ts(h, 512)], ps_ps[h][:], AF.Tanh)
    nc.sync.dma_start(out=out_d[:, :], in_=out_sb[:])


def build_program():
    nc = bacc.Bacc("TRN2", target_bir_lowering=False, debug=False,
                   enable_asserts=False)
    ctx_d = nc.dram_tensor("ctx", [BP, N, D], F32, kind="ExternalInput").ap()
    orep_d = nc.dram_tensor("orep", [BP, 128, D], F32, kind="ExternalInput").ap()
    oT_d = nc.dram_tensor("oT", [DC, 128, BP], F32, kind="ExternalInput").ap()
    WT_d = nc.dram_tensor("WT", [2 * D, D], F32, kind="ExternalInput").ap()
    brow_d = nc.dram_tensor("brow", [1, D], F32, kind="ExternalInput").ap()
    ident_d = nc.dram_tensor("ident", [128, 128], F32, kind="ExternalInput").ap()
    onesr_d = nc.dram_tensor("onesr", [1, 128], F32, kind="ExternalInput").ap()
    onesc_d = nc.dram_tensor("onesc", [128, 1], F32, kind="ExternalInput").ap()
    out_d = nc.dram_tensor("out", [BP, D], F32, kind="ExternalOutput").ap()
    attn_d = nc.dram_tensor("attn", [BP, NT, 128], F32, kind="ExternalOutput").ap()

    with tile.TileContext(nc) as tc:
        attn_kernel(tc, ctx_d, orep_d, oT_d, WT_d, brow_d, ident_d, onesr_d,
                    onesc_d, out_d, attn_d)
    nc.finalize()
    return nc


_prog_cache = {}


def _get_program():
    if "nc" not in _prog_cache:
        _prog_cache["nc"] = build_program()
    return _prog_cache["nc"]


def make_in_maps(output, context, W_out, b_out):
    o2 = np.ascontiguousarray(np.asarray(output).reshape(B, D)).astype(np.float32)
    WT = np.ascontiguousarray(np.asarray(W_out).T).astype(np.float32)
    brow = np.ascontiguousarray(np.asarray(b_out).reshape(1, D)).astype(np.float32)
    ident = np.eye(128, dtype=np.float32)
    onesr = np.ones((1, 128), np.float32)
    onesc = np.ones((128, 1), np.float32)
    in_maps = []
    for i in range(N_CORES):
        sl = slice(i * BP, (i + 1) * BP)
        osh = o2[sl]  # [BP, D]
        in_maps.append({
            "ctx": np.ascontiguousarray(np.asarray(context)[sl], dtype=np.float32),
            "orep": np.ascontiguousarray(
                np.broadcast_to(osh[:, None, :], (BP, 128, D))),
            "oT": np.ascontiguousarray(
                osh.T.reshape(DC, 128, BP)),
            "WT": WT,
            "brow": brow,
            "ident": ident,
            "onesr": onesr,
            "onesc": onesc,
        })
    return in_maps


def kernel(output, context, W_out, b_out):
    nc = _get_program()
    in_maps = make_in_maps(output, context, W_out, b_out)
    res = run_bass_kernel_spmd(nc, in_maps, list(range(N_CORES))).results
    out_full = np.empty((B, 1, D), np.float32)
    attn_full = np.empty((B, 1, N), np.float32)
    for i in range(N_CORES):
        sl = slice(i * BP, (i + 1) * BP)
        out_full[sl, 0, :] = res[i]["out"]
        attn_full[sl, 0, :] = res[i]["attn"].reshape(BP, N)
    return (out_full, attn_full)


if __name__ == "__main__":
    np.random.seed(0)
    o = np.random.randn(B, 1, D).astype(np.float32)
    c = np.random.randn(B, N, D).astype(np.float32)
    W = np.random.randn(D, 2 * D).astype(np.float32) / np.sqrt(2 * D)
    bb = np.zeros(D, np.float32)
    out, attn = kernel(output=o, context=c, W_out=W, b_out=bb)
    print(out.shape, attn.shape, out.dtype, attn.dtype)


# revision 31
# speedup vs baseline: 1.2462x; 1.0456x over previous
"""Trainium2 Bass kernel for single-query attention + output projection.

Math (per batch b):
    s   = ctx[b] @ o[b]               # (2048,)  scores
    a   = softmax(s)                  # (2048,)  attn  (output #2)
    mix = a @ ctx[b]                  # (1024,)
    out = tanh(W @ [mix; o[b]] + bias)  # (1024,)  (output #1)

Sharding: data-parallel over batch. 8 cores x 8 batches each.
Single streaming pass over context (memory-bound roofline).

Per-core pipeline:
  - ctx[b] streamed as 16 tiles [128(n), 1024(d)] (native layout).
  - scores: DVE elementwise multiply against the host-replicated query
    o_rep[b] [128, 1024], free-dim reduce on ACT via
    activation(Identity, accum_out=...). (tensor_tensor_reduce would fuse
    both, but that instruction hangs the device on this runtime.)
  - softmax: global max via PE transpose + DVE reduce + PE bcast-matmul;
    exp on ACT with fused accum_out row-sums; partition-sum via PE
    ones-matmul; reciprocal on DVE.
  - mix via PE on UNNORMALIZED exp weights P, started right after the
    exp: P column [128, 1] as stationary (tiny LDW), ctx tile halves
    [128, 512] as wide fp32 moving operand -> psum rows [1, 512],
    accumulated over n-tiles; 1/Z is folded into the PSUM->SBUF drain.
    (fp32 matmuls lower to 2 passes; a large fp32 stationary per tile
    costs ~4x more PE time than this moving-form.)
  - mix row -> combined^T columns via [1, 128] PE transposes.
  - final projection batched over the core's 8 batches:
    out[8, 1024] = tanh(combT^T @ WT + 1 x bias_row), with the bias folded
    in as an extra K=1 matmul. WT = W.T is pre-transposed on the host.

All constants (identity for PE transpose, ones vectors) are shipped from
the host; no gpsimd ops, no rearranged DRAM access patterns.
"""

import sys
from contextlib import ExitStack

import numpy as np

sys.path.insert(0, "/opt/trn_rl_repo")

import concourse.bass as bass
import concourse.tile as tile
from concourse import bacc, mybir
from concourse._compat import with_exitstack
from concourse.bass_utils import run_bass_kernel_spmd

F32 = mybir.dt.float32
AX = mybir.AluOpType
AF = mybir.ActivationFunctionType

N_CORES = 8
B, N, D = 64, 2048, 1024
BP = B // N_CORES          # batches per core = 8
NT = N // 128              # n tiles per batch = 16
DC = D // 128              # d chunks = 8
EC = 2 * D // 128          # e chunks for projection = 16


@with_exitstack
def attn_kernel(ctx: ExitStack, tc: tile.TileContext,
                ctx_d, orep_d, oT_d, WT_d, brow_d, ident_d, onesr_d, onesc_d,
                out_d, attn_d):
    nc = tc.nc

    const_pool = ctx.enter_context(tc.tile_pool(name="const", bufs=1))
    wt_pool = ctx.enter_context(tc.tile_pool(name="wt", bufs=6))
    ctx_pool = ctx.enter_context(tc.tile_pool(name="ctx", bufs=24))
    orep_pool = ctx.enter_context(tc.tile_pool(name="orep", bufs=2))
    sm_pool = ctx.enter_context(tc.tile_pool(name="sm", bufs=3))
    comb_pool = ctx.enter_context(tc.tile_pool(name="comb", bufs=1))
    scratch_pool = ctx.enter_context(tc.tile_pool(name="scratch", bufs=2))
    out_pool = ctx.enter_context(tc.tile_pool(name="outp", bufs=1))

    ps_small = ctx.enter_context(tc.tile_pool(name="ps_small", bufs=4, space="PSUM"))
    ps_mix = ctx.enter_context(tc.tile_pool(name="ps_mix", bufs=1, space="PSUM"))
    ps_proj = ctx.enter_context(tc.tile_pool(name="ps_proj", bufs=1, space="PSUM"))

    # --- constants from host ---
    ident = const_pool.tile([128, 128], F32)
    nc.sync.dma_start(out=ident[:], in_=ident_d[:, :])
    ones_row = const_pool.tile([1, 128], F32)
    nc.sync.dma_start(out=ones_row[:], in_=onesr_d[:, :])
    ones_col = const_pool.tile([128, 1], F32)
    nc.sync.dma_start(out=ones_col[:], in_=onesc_d[:, :])
    brow = const_pool.tile([1, D], F32)
    nc.sync.dma_start(out=brow[:], in_=brow_d[:, :])

    # combined^T [128, EC, BP]: e-chunks 0..7 = mix^T, 8..15 = o^T
    combT = comb_pool.tile([128, EC, BP], F32)
    for c in range(DC):
        nc.sync.dma_start(out=combT[:, DC + c, :], in_=oT_d[c])

    # Projection PSUM opens early: the o^T half of combT is ready at
    # program start, so its 8 e-chunks accumulate one-per-batch inside
    # the loop (filling PE idle gaps); only the mix half runs in the tail.
    ps_ps = [ps_proj.tile([BP, 512], F32, tag=f"ps_p{h}", name=f"ps_p{h}")
             for h in range(2)]

    for b in range(BP):
        e_early = DC + b
        wt_e = wt_pool.tile([128, D], F32, tag="wt")
        nc.sync.dma_start(out=wt_e[:], in_=WT_d[bass.ts(e_early, 128), :])
        for h in range(2):
            nc.tensor.matmul(ps_ps[h][:], combT[:, e_early, :],
                             wt_e[:, bass.ts(h, 512)],
                             start=(b == 0), stop=False)
        orep = orep_pool.tile([128, D], F32, tag="orep")
        nc.sync.dma_start(out=orep[:], in_=orep_d[b])

        # --- scores: S[p, t] = sum_d ctx[128t+p, d] * o[d] ---
        S = sm_pool.tile([128, NT], F32, tag="S")
        ctx_tiles = []
        for t in range(NT):
            ct = ctx_pool.tile([128, D], F32, tag="ctx")
            nc.sync.dma_start(out=ct[:], in_=ctx_d[b, bass.ts(t, 128), :])
            ctx_tiles.append(ct)
            scr = scratch_pool.tile([128, D], F32, tag="scr")
            nc.vector.tensor_tensor(out=scr[:], in0=ct[:], in1=orep[:],
                                    op=AX.mult)
            scr2 = scratch_pool.tile([128, D], F32, tag="scr2")
            nc.scalar.activation(scr2[:], scr[:], AF.Identity,
                                 accum_out=S[:, t:t + 1])

        # --- softmax over all 2048 scores ---
        m1 = sm_pool.tile([128, 1], F32, tag="m1")
        nc.vector.reduce_max(m1[:], S[:], axis=mybir.AxisListType.X)
        ps_tr = ps_small.tile([1, 128], F32, tag="ps_small")
        nc.tensor.transpose(ps_tr[:], m1[:], ident[:])
        mm = sm_pool.tile([1, 1], F32, tag="mm")
        nc.vector.reduce_max(mm[:], ps_tr[:], axis=mybir.AxisListType.X)
        ps_bc = ps_small.tile([128, 1], F32, tag="ps_small")
        nc.tensor.matmul(ps_bc[:], ones_row[:], mm[:], start=True, stop=True)
        mneg = sm_pool.tile([128, 1], F32, tag="mneg")
        nc.scalar.mul(mneg[:], ps_bc[:], -1.0)

        P = sm_pool.tile([128, NT], F32, tag="P")
        rs = sm_pool.tile([128, 1], F32, tag="rs")
        nc.scalar.activation(P[:], S[:], AF.Exp, bias=mneg[:], scale=1.0,
                             accum_out=rs[:])
        ps_sum = ps_small.tile([1, 1], F32, tag="ps_small")
        nc.tensor.matmul(ps_sum[:], ones_col[:], rs[:], start=True, stop=True)

        # --- mix immediately on UNNORMALIZED exp weights P (1/Z folded in
        # at the PSUM->SBUF copy): keeps PE busy while the sum/reciprocal/
        # broadcast chain and the attn-output path run on other engines.
        psr = [ps_mix.tile([1, 512], F32, tag=f"psr{h}", name=f"psr{h}")
               for h in range(2)]
        for t in range(NT):
            for h in range(2):
                nc.tensor.matmul(psr[h][:], P[:, t:t + 1],
                                 ctx_tiles[t][:, bass.ts(h, 512)],
                                 start=(t == 0), stop=(t == NT - 1))

        inv = sm_pool.tile([1, 1], F32, tag="inv")
        nc.vector.reciprocal(inv[:], ps_sum[:])
        ps_bi = ps_small.tile([128, 1], F32, tag="ps_small")
        nc.tensor.matmul(ps_bi[:], ones_row[:], inv[:], start=True, stop=True)
        inv_col = sm_pool.tile([128, 1], F32, tag="inv_col")
        nc.scalar.copy(inv_col[:], ps_bi[:])

        A = sm_pool.tile([128, NT], F32, tag="A")
        nc.vector.tensor_scalar_mul(A[:], P[:], inv_col[:])

        # --- attn output: transpose [128, 16] -> [16, 128] for contiguous DMA ---
        ps_at = ps_small.tile([NT, 128], F32, tag="ps_small")
        nc.tensor.transpose(ps_at[:], A[:], ident[:])
        At = sm_pool.tile([NT, 128], F32, tag="At")
        nc.scalar.copy(At[:], ps_at[:])
        nc.sync.dma_start(out=attn_d[b], in_=At[:])

        mix_row = sm_pool.tile([1, D], F32, tag="mix_row")
        for h in range(2):
            nc.vector.tensor_scalar_mul(mix_row[:, bass.ts(h, 512)],
                                        psr[h][:], inv[:])
        # row -> combT columns via PE transpose of [1, 128] slices
        for c in range(DC):
            ps_mc = ps_small.tile([128, 1], F32, tag="ps_small")
            nc.tensor.transpose(ps_mc[:], mix_row[:, bass.ts(c, 128)],
                                ident[0:1, 0:1])
            nc.scalar.copy(combT[:, c:c + 1, b], ps_mc[:])

    # --- projection tail: mix-part e-chunks + bias, then tanh ---
    out_sb = out_pool.tile([BP, D], F32, tag="out_sb")
    for e in range(DC):
        wt = wt_pool.tile([128, D], F32, tag="wt")
        nc.sync.dma_start(out=wt[:], in_=WT_d[bass.ts(e, 128), :])
        for h in range(2):
            nc.tensor.matmul(ps_ps[h][:], combT[:, e, :],
                             wt[:, bass.ts(h, 512)],
                             start=False, stop=False)
    for h in range(2):
        nc.tensor.matmul(ps_ps[h][:], ones_row[:1, 0:BP],
                         brow[:, bass.ts(h, 512)], start=False, stop=True)
        nc.scalar.activation(out_sb[:, bass.ts(h, 512)], ps_ps[h][:], AF.Tanh)
    nc.sync.dma_start(out=out_d[:, :], in_=out_sb[:])


def build_program():
    nc = bacc.Bacc("TRN2", target_bir_lowering=False, debug=False,
                   enable_asserts=False)
    ctx_d = nc.dram_tensor("ctx", [BP, N, D], F32, kind="ExternalInput").ap()
    orep_d = nc.dram_tensor("orep", [BP, 128, D], F32, kind="ExternalInput").ap()
    oT_d = nc.dram_tensor("oT", [DC, 128, BP], F32, kind="ExternalInput").ap()
    WT_d = nc.dram_tensor("WT", [2 * D, D], F32, kind="ExternalInput").ap()
    brow_d = nc.dram_tensor("brow", [1, D], F32, kind="ExternalInput").ap()
    ident_d = nc.dram_tensor("ident", [128, 128], F32, kind="ExternalInput").ap()
    onesr_d = nc.dram_tensor("onesr", [1, 128], F32, kind="ExternalInput").ap()
    onesc_d = nc.dram_tensor("onesc", [128, 1], F32, kind="ExternalInput").ap()
    out_d = nc.dram_tensor("out", [BP, D], F32, kind="ExternalOutput").ap()
    attn_d = nc.dram_tensor("attn", [BP, NT, 128], F32, kind="ExternalOutput").ap()

    with tile.TileContext(nc) as tc:
        attn_kernel(tc, ctx_d, orep_d, oT_d, WT_d, brow_d, ident_d, onesr_d,
                    onesc_d, out_d, attn_d)
    nc.finalize()
    return nc


_prog_cache = {}


def _get_program():
    if "nc" not in _prog_cache:
        _prog_cache["nc"] = build_program()
    return _prog_cache["nc"]


def make_in_maps(output, context, W_out, b_out):
    o2 = np.ascontiguousarray(np.asarray(output).reshape(B, D)).astype(np.float32)
    WT = np.ascontiguousarray(np.asarray(W_out).T).astype(np.float32)
    brow = np.ascontiguousarray(np.asarray(b_out).reshape(1, D)).astype(np.float32)
    ident = np.eye(128, dtype=np.float32)
    onesr = np.ones((1, 128), np.float32)
    onesc = np.ones((128, 1), np.float32)
    in_maps = []
    for i in range(N_CORES):
        sl = slice(i * BP, (i + 1) * BP)
        osh = o2[sl]  # [BP, D]
        in_maps.append({
            "ctx": np.ascontiguousarray(np.asarray(context)[sl], dtype=np.float32),
            "orep": np.ascontiguousarray(
                np.broadcast_to(osh[:, None, :], (BP, 128, D))),
            "oT": np.ascontiguousarray(
                osh.T.reshape(DC, 128, BP)),
            "WT": WT,
            "brow": brow,
            "ident": ident,
            "onesr": onesr,
            "onesc": onesc,
        })
    return in_maps


def kernel(output, context, W_out, b_out):
    nc = _get_program()
    in_maps = make_in_maps(output, context, W_out, b_out)
    res = run_bass_kernel_spmd(nc, in_maps, list(range(N_CORES))).results
    out_full = np.empty((B, 1, D), np.float32)
    attn_full = np.empty((B, 1, N), np.float32)
    for i in range(N_CORES):
        sl = slice(i * BP, (i + 1) * BP)
        out_full[sl, 0, :] = res[i]["out"]
        attn_full[sl, 0, :] = res[i]["attn"].reshape(BP, N)
    return (out_full, attn_full)


if __name__ == "__main__":
    np.random.seed(0)
    o = np.random.randn(B, 1, D).astype(np.float32)
    c = np.random.randn(B, N, D).astype(np.float32)
    W = np.random.randn(D, 2 * D).astype(np.float32) / np.sqrt(2 * D)
    bb = np.zeros(D, np.float32)
    out, attn = kernel(output=o, context=c, W_out=W, b_out=bb)
    print(out.shape, attn.shape, out.dtype, attn.dtype)


# revision 37
# speedup vs baseline: 1.2541x; 1.0063x over previous
"""Trainium2 Bass kernel for single-query attention + output projection.

Math (per batch b):
    s   = ctx[b] @ o[b]               # (2048,)  scores
    a   = softmax(s)                  # (2048,)  attn  (output #2)
    mix = a @ ctx[b]                  # (1024,)
    out = tanh(W @ [mix; o[b]] + bias)  # (1024,)  (output #1)

Sharding: data-parallel over batch. 8 cores x 8 batches each.
Single streaming pass over context (memory-bound roofline).

Per-core pipeline:
  - ctx[b] streamed as 16 tiles [128(n), 1024(d)] (native layout).
  - scores: DVE elementwise multiply against the host-replicated query
    o_rep[b] [128, 1024], free-dim reduce on ACT via
    activation(Identity, accum_out=...). (tensor_tensor_reduce would fuse
    both, but that instruction hangs the device on this runtime.)
  - softmax: global max via PE transpose + DVE reduce + PE bcast-matmul;
    exp on ACT with fused accum_out row-sums; partition-sum via PE
    ones-matmul; reciprocal on DVE.
  - mix via PE on UNNORMALIZED exp weights P, started right after the
    exp: P column [128, 1] as stationary (tiny LDW), ctx tile halves
    [128, 512] as wide fp32 moving operand -> psum rows [1, 512],
    accumulated over n-tiles; 1/Z is folded into the PSUM->SBUF drain.
    (fp32 matmuls lower to 2 passes; a large fp32 stationary per tile
    costs ~4x more PE time than this moving-form.)
  - mix row -> combined^T columns via [1, 128] PE transposes.
  - final projection batched over the core's 8 batches:
    out[8, 1024] = tanh(combT^T @ WT + 1 x bias_row), with the bias folded
    in as an extra K=1 matmul. WT = W.T is pre-transposed on the host.

All constants (identity for PE transpose, ones vectors) are shipped from
the host; no gpsimd ops, no rearranged DRAM access patterns.
"""

import sys
from contextlib import ExitStack

import numpy as np

sys.path.insert(0, "/opt/trn_rl_repo")

import concourse.bass as bass
import concourse.tile as tile
from concourse import bacc, mybir
from concourse._compat import with_exitstack
from concourse.bass_utils import run_bass_kernel_spmd

F32 = mybir.dt.float32
AX = mybir.AluOpType
AF = mybir.ActivationFunctionType

N_CORES = 8
B, N, D = 64, 2048, 1024
BP = B // N_CORES          # batches per core = 8
NT = N // 128              # n tiles per batch = 16
DC = D // 128              # d chunks = 8
EC = 2 * D // 128          # e chunks for projection = 16


@with_exitstack
def attn_kernel(ctx: ExitStack, tc: tile.TileContext,
                ctx_d, orep_d, oT_d, WT_d, brow_d, ident_d, onesr_d, onesc_d,
                out_d, attn_d):
    nc = tc.nc

    const_pool = ctx.enter_context(tc.tile_pool(name="const", bufs=1))
    wt_pool = ctx.enter_context(tc.tile_pool(name="wt", bufs=3))
    ctx_pool = ctx.enter_context(tc.tile_pool(name="ctx", bufs=24))
    orep_pool = ctx.enter_context(tc.tile_pool(name="orep", bufs=3))
    sm_pool = ctx.enter_context(tc.tile_pool(name="sm", bufs=3))
    comb_pool = ctx.enter_context(tc.tile_pool(name="comb", bufs=1))
    scratch_pool = ctx.enter_context(tc.tile_pool(name="scratch", bufs=2))
    out_pool = ctx.enter_context(tc.tile_pool(name="outp", bufs=1))

    ps_small = ctx.enter_context(tc.tile_pool(name="ps_small", bufs=4, space="PSUM"))
    ps_mix = ctx.enter_context(tc.tile_pool(name="ps_mix", bufs=1, space="PSUM"))
    ps_proj = ctx.enter_context(tc.tile_pool(name="ps_proj", bufs=1, space="PSUM"))

    # --- constants from host ---
    ident = const_pool.tile([128, 128], F32)
    nc.sync.dma_start(out=ident[:], in_=ident_d[:, :])
    ones_row = const_pool.tile([1, 128], F32)
    nc.sync.dma_start(out=ones_row[:], in_=onesr_d[:, :])
    ones_col = const_pool.tile([128, 1], F32)
    nc.sync.dma_start(out=ones_col[:], in_=onesc_d[:, :])
    brow = const_pool.tile([1, D], F32)
    nc.sync.dma_start(out=brow[:], in_=brow_d[:, :])

    # combined^T [128, EC, BP]: e-chunks 0..7 = mix^T, 8..15 = o^T
    combT = comb_pool.tile([128, EC, BP], F32)
    for c in range(DC):
        nc.sync.dma_start(out=combT[:, DC + c, :], in_=oT_d[c])

    # Projection PSUM opens early: the o^T half of combT is ready at
    # program start, so its 8 e-chunks accumulate one-per-batch inside
    # the loop (filling PE idle gaps); only the mix half runs in the tail.
    ps_ps = [ps_proj.tile([BP, 512], F32, tag=f"ps_p{h}", name=f"ps_p{h}")
             for h in range(2)]

    for b in range(BP):
        e_early = DC + b
        wt_e = wt_pool.tile([128, D], F32, tag="wt")
        nc.sync.dma_start(out=wt_e[:], in_=WT_d[bass.ts(e_early, 128), :])
        for h in range(2):
            nc.tensor.matmul(ps_ps[h][:], combT[:, e_early, :],
                             wt_e[:, bass.ts(h, 512)],
                             start=(b == 0), stop=False)
        orep = orep_pool.tile([128, D], F32, tag="orep")
        nc.sync.dma_start(out=orep[:], in_=orep_d[b])

        # --- scores: S[p, t] = sum_d ctx[128t+p, d] * o[d] ---
        S = sm_pool.tile([128, NT], F32, tag="S")
        ctx_tiles = []
        for t in range(NT):
            ct = ctx_pool.tile([128, D], F32, tag="ctx")
            nc.sync.dma_start(out=ct[:], in_=ctx_d[b, bass.ts(t, 128), :])
            ctx_tiles.append(ct)
            scr = scratch_pool.tile([128, D], F32, tag="scr")
            nc.vector.tensor_tensor(out=scr[:], in0=ct[:], in1=orep[:],
                                    op=AX.mult)
            scr2 = scratch_pool.tile([128, D], F32, tag="scr2")
            nc.scalar.activation(scr2[:], scr[:], AF.Identity,
                                 accum_out=S[:, t:t + 1])

        # --- softmax over all 2048 scores ---
        m1 = sm_pool.tile([128, 1], F32, tag="m1")
        nc.vector.reduce_max(m1[:], S[:], axis=mybir.AxisListType.X)
        ps_tr = ps_small.tile([1, 128], F32, tag="ps_small")
        nc.tensor.transpose(ps_tr[:], m1[:], ident[:])
        mm = sm_pool.tile([1, 1], F32, tag="mm")
        nc.vector.reduce_max(mm[:], ps_tr[:], axis=mybir.AxisListType.X)
        ps_bc = ps_small.tile([128, 1], F32, tag="ps_small")
        nc.tensor.matmul(ps_bc[:], ones_row[:], mm[:], start=True, stop=True)
        mneg = sm_pool.tile([128, 1], F32, tag="mneg")
        nc.scalar.mul(mneg[:], ps_bc[:], -1.0)

        P = sm_pool.tile([128, NT], F32, tag="P")
        rs = sm_pool.tile([128, 1], F32, tag="rs")
        nc.scalar.activation(P[:], S[:], AF.Exp, bias=mneg[:], scale=1.0,
                             accum_out=rs[:])
        ps_sum = ps_small.tile([1, 1], F32, tag="ps_small")
        nc.tensor.matmul(ps_sum[:], ones_col[:], rs[:], start=True, stop=True)

        # --- mix immediately on UNNORMALIZED exp weights P (1/Z folded in
        # at the PSUM->SBUF copy): keeps PE busy while the sum/reciprocal/
        # broadcast chain and the attn-output path run on other engines.
        psr = [ps_mix.tile([1, 512], F32, tag=f"psr{h}", name=f"psr{h}")
               for h in range(2)]
        for t in range(NT):
            for h in range(2):
                nc.tensor.matmul(psr[h][:], P[:, t:t + 1],
                                 ctx_tiles[t][:, bass.ts(h, 512)],
                                 start=(t == 0), stop=(t == NT - 1))

        inv = sm_pool.tile([1, 1], F32, tag="inv")
        nc.vector.reciprocal(inv[:], ps_sum[:])
        ps_bi = ps_small.tile([128, 1], F32, tag="ps_small")
        nc.tensor.matmul(ps_bi[:], ones_row[:], inv[:], start=True, stop=True)
        inv_col = sm_pool.tile([128, 1], F32, tag="inv_col")
        nc.scalar.copy(inv_col[:], ps_bi[:])

        A = sm_pool.tile([128, NT], F32, tag="A")
        nc.vector.tensor_scalar_mul(A[:], P[:], inv_col[:])

        # --- attn output: transpose [128, 16] -> [16, 128] for contiguous DMA ---
        ps_at = ps_small.tile([NT, 128], F32, tag="ps_small")
        nc.tensor.transpose(ps_at[:], A[:], ident[:])
        At = sm_pool.tile([NT, 128], F32, tag="At")
        nc.scalar.copy(At[:], ps_at[:])
        nc.sync.dma_start(out=attn_d[b], in_=At[:])

        mix_row = sm_pool.tile([1, D], F32, tag="mix_row")
        for h in range(2):
            nc.vector.tensor_scalar_mul(mix_row[:, bass.ts(h, 512)],
                                        psr[h][:], inv[:])
        # row -> combT columns via PE transpose of [1, 128] slices
        for c in range(DC):
            ps_mc = ps_small.tile([128, 1], F32, tag="ps_small")
            nc.tensor.transpose(ps_mc[:], mix_row[:, bass.ts(c, 128)],
                                ident[0:1, 0:1])
            nc.scalar.copy(combT[:, c:c + 1, b], ps_mc[:])

    # --- projection tail: mix-part e-chunks + bias, then tanh ---
    out_sb = out_pool.tile([BP, D], F32, tag="out_sb")
    for e in range(DC):
        wt = wt_pool.tile([128, D], F32, tag="wt")
        nc.sync.dma_start(out=wt[:], in_=WT_d[bass.ts(e, 128), :])
        for h in range(2):
            nc.tensor.matmul(ps_ps[h][:], combT[:, e, :],
                             wt[:, bass.ts(h, 512)],
                             start=False, stop=False)
    for h in range(2):
        nc.tensor.matmul(ps_ps[h][:], ones_row[:1, 0:BP],
                         brow[:, bass.ts(h, 512)], start=False, stop=True)
        nc.scalar.activation(out_sb[:, bass.ts(h, 512)], ps_ps[h][:], AF.Tanh)
    nc.sync.dma_start(out=out_d[:, :], in_=out_sb[:])


def build_program():
    nc = bacc.Bacc("TRN2", target_bir_lowering=False, debug=False,
                   enable_asserts=False)
    ctx_d = nc.dram_tensor("ctx", [BP, N, D], F32, kind="ExternalInput").ap()
    orep_d = nc.dram_tensor("orep", [BP, 128, D], F32, kind="ExternalInput").ap()
    oT_d = nc.dram_tensor("oT", [DC, 128, BP], F32, kind="ExternalInput").ap()
    WT_d = nc.dram_tensor("WT", [2 * D, D], F32, kind="ExternalInput").ap()
    brow_d = nc.dram_tensor("brow", [1, D], F32, kind="ExternalInput").ap()
    ident_d = nc.dram_tensor("ident", [128, 128], F32, kind="ExternalInput").ap()
    onesr_d = nc.dram_tensor("onesr", [1, 128], F32, kind="ExternalInput").ap()
    onesc_d = nc.dram_tensor("onesc", [128, 1], F32, kind="ExternalInput").ap()
    out_d = nc.dram_tensor("out", [BP, D], F32, kind="ExternalOutput").ap()
    attn_d = nc.dram_tensor("attn", [BP, NT, 128], F32, kind="ExternalOutput").ap()

    with tile.TileContext(nc) as tc:
        attn_kernel(tc, ctx_d, orep_d, oT_d, WT_d, brow_d, ident_d, onesr_d,
                    onesc_d, out_d, attn_d)
    nc.finalize()
    return nc


_prog_cache = {}


def _get_program():
    if "nc" not in _prog_cache:
        _prog_cache["nc"] = build_program()
    return _prog_cache["nc"]


def make_in_maps(output, context, W_out, b_out):
    o2 = np.ascontiguousarray(np.asarray(output).reshape(B, D)).astype(np.float32)
    WT = np.ascontiguousarray(np.asarray(W_out).T).astype(np.float32)
    brow = np.ascontiguousarray(np.asarray(b_out).reshape(1, D)).astype(np.float32)
    ident = np.eye(128, dtype=np.float32)
    onesr = np.ones((1, 128), np.float32)
    onesc = np.ones((128, 1), np.float32)
    in_maps = []
    for i in range(N_CORES):
        sl = slice(i * BP, (i + 1) * BP)
        osh = o2[sl]  # [BP, D]
        in_maps.append({
            "ctx": np.ascontiguousarray(np.asarray(context)[sl], dtype=np.float32),
            "orep": np.ascontiguousarray(
                np.broadcast_to(osh[:, None, :], (BP, 128, D))),
            "oT": np.ascontiguousarray(
                osh.T.reshape(DC, 128, BP)),
            "WT": WT,
            "brow": brow,
            "ident": ident,
            "onesr": onesr,
            "onesc": onesc,
        })
    return in_maps


def kernel(output, context, W_out, b_out):
    nc = _get_program()
    in_maps = make_in_maps(output, context, W_out, b_out)
    res = run_bass_kernel_spmd(nc, in_maps, list(range(N_CORES))).results
    out_full = np.empty((B, 1, D), np.float32)
    attn_full = np.empty((B, 1, N), np.float32)
    for i in range(N_CORES):
        sl = slice(i * BP, (i + 1) * BP)
        out_full[sl, 0, :] = res[i]["out"]
        attn_full[sl, 0, :] = res[i]["attn"].reshape(BP, N)
    return (out_full, attn_full)


if __name__ == "__main__":
    np.random.seed(0)
    o = np.random.randn(B, 1, D).astype(np.float32)
    c = np.random.randn(B, N, D).astype(np.float32)
    W = np.random.randn(D, 2 * D).astype(np.float32) / np.sqrt(2 * D)
    bb = np.zeros(D, np.float32)
    out, attn = kernel(output=o, context=c, W_out=W, b_out=bb)
    print(out.shape, attn.shape, out.dtype, attn.dtype)
